# revision 13
# baseline (speedup 1.0000x reference)
"""MEX (log-sum-exp) 3x3 pooling kernel for Trainium2, 8-core SPMD.

Math: out[b,m,i,j] = log( (1/n) * sum_{c,dh,dw} exp(x[b,c,i+dh,j+dw] + off[m,c,dh,dw]) )
with n = C*3*3 = 576, eps = 1.

Identity used: the reference's per-pixel max-stabilization cancels exactly:
  out = log( sum_k exp(x_k + b_k) ) - log(n);  values are benign in fp32.

Per-core plan (core i handles batch images 2i, 2i+1):
  - E[p = img*64+c, h*128+w] = exp(x) fp16, streamed in chunks (ACT).
  - LT[p, dh*96 + dw*32 + img*16 + m] = exp(off + alpha) fp16, block-diagonal
    over img.
  - Superchunk s (2048 px = 16 output rows): 12 PSUM-accumulated matmuls
    (3 dh x 4 banks of 512) -> P[(dw,img,m), pix] per-dw-tap partial sums.
  - Evacuate: ONE DVE copy P[96,2048] -> batch buffer CB bf16 in SBUF.  Cost
    is free-dim-bound, so 96 rows cost the same as 32.
  - Fold (per batch of superchunks): 12 SBUF->SBUF DMAs remap the three
    32-row dw groups into F0/F1/F2[128 = 4x32 rows, FW] with the dw column
    shifts baked into src offsets.  DMA is the only partition-mover; each
    ring serializes DMAs at ~0.65us fixed cost, so batches are few and the
    12 DMAs are spread across the SWDGE and SP rings.
  - Combine: S = F0+F1+F2 via two DVE bf16 adds at 128 partitions (2x mode).
  - ACT Ln on [128, FW] (4x less FD than a 32-partition log); out-DMA on the
    ACT ring right after its Ln (no stall), to a permuted DRAM layout that
    the host decodes (host does reshape only).
"""

import math as _math
import os as _os

import numpy as np

EPS = 1.0
B, C, H, W = 16, 64, 128, 128
M = 16
BH = BW = 3
HO, WO = H - BH + 1, W - BW + 1  # 126, 126
N_TAPS = C * BH * BW  # 576
NCORES = 8
BPC = B // NCORES  # 2 images per core
HWP = H * W  # 16384 pixels per image plane
PAD = 768
SC = 2048  # superchunk pixels (4 psum banks of 512 fp32)
NSC = HWP // SC  # 8
# fold batches: (first superchunk, num superchunks)
BATCHES = [(0, 5), (5, 2), (7, 1)]

MM_DTYPE = _os.environ.get("MEX_MM_DTYPE", "f16")
W_ALPHA_LOG = {"f16": 10.0 * 0.6931471805599453}.get(MM_DTYPE, 0.0)

_BUILT = {}


def _build(mm_dtype: str):
    """Build (and cache) the Bass/Tile program shared by all 8 cores."""
    if mm_dtype in _BUILT:
        return _BUILT[mm_dtype]

    import concourse.bass as bass
    import concourse.bacc as bacc
    import concourse.tile as tile
    from concourse import mybir

    f32 = mybir.dt.float32
    bf16 = mybir.dt.bfloat16
    mdt = {
        "f32r": mybir.dt.float32r,
        "f32": f32,
        "bf16": mybir.dt.bfloat16,
        "f16": mybir.dt.float16,
    }[mm_dtype]
    w_alpha_log = 10.0 * _math.log(2.0) if mm_dtype == "f16" else 0.0
    AF = mybir.ActivationFunctionType

    nc = bacc.Bacc("TRN2", target_bir_lowering=False, debug=False)

    # Preload the ACT table set that contains BOTH exp and ln so the
    # interleaved exp/Ln stream never thrashes table loads (~1.3us each).
    from concourse.hw_specs import get_activation_tables

    _tabs = list(get_activation_tables(nc.m.arch).keys())
    _combined_id = _tabs.index("natural_log_exp_and_others")

    xd = nc.dram_tensor("x", [128, HWP], f32, kind="ExternalInput")
    wpd = nc.dram_tensor("wp", [64, 144], f32, kind="ExternalInput")
    outd = nc.dram_tensor("out", [128, HWP // 4], f32, kind="ExternalOutput")

    with tile.TileContext(nc) as tc:
        with (
            tc.tile_pool(name="singles", bufs=1) as singles,
            tc.tile_pool(name="xin", bufs=6) as xin,
            tc.tile_pool(name="psum", bufs=2, space="PSUM") as psum,
            tc.tile_pool(name="post", bufs=2) as post,
        ):
            nc.scalar.add_instruction(
                mybir.InstLoadActFuncSet(
                    name=nc.get_next_instruction_name(),
                    act_func_set_id=_combined_id,
                    ins=[],
                    outs=[],
                )
            )
            # ---- weights DMA first on the SP ring, then the x stream. ----
            Q = singles.tile([128, 144], f32)
            nc.sync.dma_start(out=Q[0:64, :], in_=bass.AP(wpd, 0, [[144, 64], [1, 144]]))
            nc.sync.dma_start(out=Q[64:128, :], in_=bass.AP(wpd, 0, [[144, 64], [1, 144]]))
            QL = singles.tile([128, 288], f32)
            nc.vector.memset(QL[:, :], -80.0)
            QLv = QL[:, :].rearrange("p (dh dw i m) -> p dh dw i m", dh=3, dw=3, i=2)
            Qv = Q[:, :].rearrange("p (dh dw m) -> p dh dw m", dh=3, dw=3)
            nc.vector.tensor_scalar_add(
                out=QLv[0:64, :, :, 0, :], in0=Qv[0:64], scalar1=w_alpha_log
            )
            nc.vector.tensor_scalar_add(
                out=QLv[64:128, :, :, 1, :], in0=Qv[64:128], scalar1=w_alpha_log
            )
            LT = singles.tile([128, 288], mdt)
            nc.scalar.activation(out=LT[:, :], in_=QL[:, :], func=AF.Exp, scale=EPS)

            # ---- E = exp(x); pad gets exp(0)=1 ----
            E = singles.tile([128, HWP + PAD], mdt)
            Xpad = singles.tile([128, PAD], f32)
            nc.vector.memset(Xpad[:, :], 0.0)
            nc.scalar.activation(out=E[:, HWP:], in_=Xpad[:, :], func=AF.Exp, scale=EPS)

            # batch buffers for the evacuated psum (bf16), padded so the fold
            # DMAs' +1/+2 column shifts stay in bounds.
            CB = []
            for bi, (s0, ns) in enumerate(BATCHES):
                cb = singles.tile([128, ns * SC + 8], bf16, name=f"CB{bi}")
                nc.vector.memset(cb[:, ns * SC : ns * SC + 8], 0.0)
                CB.append(cb)

            # x DMAs: the 6 small head chunks ride the otherwise-idle ACT
            # ring so the big chunks start on the SP ring immediately
            # (each ring serializes ~0.65us fixed cost per DMA).
            xchunks = [256, 256, 512, 512, 512, 512] + [2048] * 6 + [1024, 512]
            xoff = 0
            xk_tiles = []
            for ci, npx in enumerate(xchunks):
                Xk = xin.tile([128, npx], f32, tag="Xk")
                dma = nc.scalar.dma_start if ci < 6 else nc.sync.dma_start
                dma(out=Xk[:, :], in_=bass.AP(xd, xoff, [[HWP, 128], [1, npx]]))
                xk_tiles.append((Xk, xoff, npx))
                xoff += npx

            def emit_exp(k):
                Xk, xo, npx = xk_tiles[k]
                nc.scalar.activation(
                    out=E[:, xo : xo + npx], in_=Xk[:, :], func=AF.Exp, scale=EPS
                )

            xcum = []
            _c = 0
            for npx in xchunks:
                _c += npx
                xcum.append(_c)
            next_exp = 0

            LTd = LT[:, :].rearrange("p (dh c) -> p dh c", dh=3)
            ln_scale = 1.0 / (float(N_TAPS) * _math.exp(w_alpha_log))

            def emit_mains(s):
                P = psum.tile([96, SC], f32, tag="P")
                p0 = s * SC
                for dh in range(3):
                    lhsT = LTd[:, dh, :]
                    for b4 in range(SC // 512):
                        base = p0 + dh * W + b4 * 512
                        nc.tensor.matmul(
                            P[:, b4 * 512 : (b4 + 1) * 512],
                            lhsT,
                            E[:, base : base + 512],
                            start=(dh == 0),
                            stop=(dh == 2),
                        )
                return P

            fold_tiles = {}

            def emit_fold(bi, engines):
                """12 SBUF->SBUF fold DMAs for batch bi, spread over rings."""
                s0, ns = BATCHES[bi]
                fw = ns * SC // 4
                cb = CB[bi]
                F0 = post.tile([128, fw], bf16, tag="F0", name=f"F0_{bi}")
                F1 = post.tile([128, fw], bf16, tag="F1", name=f"F1_{bi}")
                F2 = post.tile([128, fw], bf16, tag="F2", name=f"F2_{bi}")
                i = 0
                for g, Fg in enumerate((F0, F1, F2)):
                    for q in range(4):
                        eng = engines[i % len(engines)]
                        i += 1
                        eng(
                            out=Fg[32 * q : 32 * q + 32, :],
                            in_=cb[32 * g : 32 * g + 32, fw * q + g : fw * q + g + fw],
                        )
                fold_tiles[bi] = (F0, F1, F2)

            def emit_adds(bi):
                s0, ns = BATCHES[bi]
                fw = ns * SC // 4
                F0, F1, F2 = fold_tiles.pop(bi)
                A = post.tile([128, fw], bf16, tag="A", name=f"A_{bi}")
                nc.vector.tensor_add(out=A[:, :], in0=F0[:, :], in1=F1[:, :])
                S = post.tile([128, fw], bf16, tag="S", name=f"S_{bi}")
                nc.vector.tensor_add(out=S[:, :], in0=A[:, :], in1=F2[:, :])
                return S

            def emit_ln_out(bi, S):
                """Ln on ACT then the out DMA on the ACT ring (no stall)."""
                s0, ns = BATCHES[bi]
                fw = ns * SC // 4
                LG = post.tile([128, fw], f32, tag="LG", name=f"LG_{bi}")
                nc.scalar.activation(out=LG[:, :], in_=S[:, :], func=AF.Ln, scale=ln_scale)
                nc.sync.dma_start(
                    out=bass.AP(outd, s0 * SC // 4, [[HWP // 4, 128], [1, fw]]),
                    in_=LG[:, :],
                )

            # batch id for each superchunk + offset within the batch
            sc2batch = {}
            for bi, (s0, ns) in enumerate(BATCHES):
                for k in range(ns):
                    sc2batch[s0 + k] = (bi, k)

            gdma = nc.gpsimd.dma_start
            sdma = nc.sync.dma_start
            for s in range(NSC):
                # ensure E coverage for this superchunk's matmuls first
                needed = min(SC * (s + 1) + 256, HWP)
                while next_exp < len(xchunks) and (
                    next_exp == 0 or xcum[next_exp - 1] < needed
                ):
                    emit_exp(next_exp)
                    next_exp += 1
                P = emit_mains(s)
                bi, k = sc2batch[s]
                cb = CB[bi]
                dst = cb[0:96, k * SC : (k + 1) * SC]
                if s in (5, 7):
                    nc.scalar.copy(out=dst, in_=P[:, :])
                else:
                    nc.vector.tensor_copy(out=dst, in_=P[:, :])
                if s == 4:
                    emit_fold(0, [gdma])  # batch A: SWDGE, hidden under x
                if s == 6:
                    emit_fold(1, [gdma])
                if s == 7:
                    emit_fold(2, [sdma, gdma])
            S0 = emit_adds(0)
            S1 = emit_adds(1)
            S2 = emit_adds(2)
            emit_ln_out(0, S0)
            emit_ln_out(1, S1)
            emit_ln_out(2, S2)

    nc.compile()
    _BUILT[mm_dtype] = nc
    return nc


def _prep_inputs(x, offsets):
    x = np.ascontiguousarray(np.asarray(x), dtype=np.float32)
    off = np.asarray(offsets, dtype=np.float32).reshape(M, C, BH, BW)
    # wp[c, dh*48 + dw*16 + m] = off[m, c, dh, dw]
    wp = np.ascontiguousarray(np.transpose(off, (1, 2, 3, 0)).reshape(64, 144))
    in_maps = [
        {"x": np.ascontiguousarray(x[BPC * i : BPC * (i + 1)]).reshape(128, HWP), "wp": wp}
        for i in range(NCORES)
    ]
    return in_maps


def _decode(raw):
    """raw [128, 4096] per core -> [BPC, M, HO, WO].

    Batch at superchunk s0 with ns superchunks has fold width fw = ns*512:
    raw[32q+p, s0*512 + fw-col j] = out[p, row 16*s0 + (fw//128)*q + j//128,
    col j%128].
    """
    a = np.empty((32, 128, 128), dtype=raw.dtype)
    for s0, ns in BATCHES:
        fw = ns * 512
        rows_per_q = fw // 128  # rows per partition group
        blk = raw[:, s0 * 512 : s0 * 512 + fw].reshape(4, 32, rows_per_q, 128)
        r0 = 16 * s0
        a[:, r0 : r0 + 4 * rows_per_q, :] = blk.transpose(1, 0, 2, 3).reshape(
            32, 4 * rows_per_q, 128
        )
    return a[:, :HO, :WO].reshape(BPC, M, HO, WO)


def kernel(x, offsets):
    from concourse.bass_utils import run_bass_kernel_spmd

    nc = _build(MM_DTYPE)
    in_maps = _prep_inputs(x, offsets)
    res = run_bass_kernel_spmd(nc, in_maps, core_ids=list(range(NCORES)))
    out = np.empty((B, M, HO, WO), dtype=np.float32)
    for i in range(NCORES):
        out[BPC * i : BPC * (i + 1)] = _decode(res.results[i]["out"])
    return out
